# revision 1
# baseline (speedup 1.0000x reference)
"""Trainium2 Bass kernel for nn_C4MoEVM (moe_routing).

Math: every softmax "lookup" in the reference is exactly one-hot in fp32
(scale=1000 => exp(-1000) underflows to 0), so the module reduces to
  opcode 0: a+b   1: a-b   2: round(a*b) == a*b (exact, <=225)
  opcode 3,4,5: a&b, a|b, a^b   (integer bitwise on 4-bit values)
  opcode 6: y0 = recip_val[idx] == fp32(1/z), z = 0.25 + (b*2^-e)/2,
            e = floor(log2 b)+1; two Newton steps y <- y*(2 - temp*y);
            recip = y * 2^-e.
Routing gates are a numerically-exact one-hot selection by opcode (off-diag
gate leakage is ~2e-9 relative — negligible under a norm metric).

Key transformations:
- Scaled Newton: with Y_n := y_n * 2^-e, the iteration becomes
  Y_{n+1} = (2 - b*Y_n)*Y_n, and Y0 = 1/Z for Z = z*2^e = 0.5*(b + 2^(e-1)).
  Power-of-two scaling commutes with fp32 rounding, so Y2 is bit-identical
  to the reference's y2*2^-e. 2^(e-1) is extracted by masking b's fp32
  mantissa (bitwise AND with the +inf bit pattern 0x7F800000).
- Sign packing: host sends b8s = -b where opcode==1 and a8s = -a where
  opcode==2. Then a+b8s covers both add and sub; a single fused DVE op
  (select on sign of a8s) also covers mul. Bitwise experts (opcodes 3-5)
  see the original positive values.
- Custom DVE ops (registered at import into concourse.dve_ops): FAM
  (fused add/sub/mul select), FASTZ (Z from b's bits), NEWTON2B (both
  Newton steps in one 6-stage instruction).

Raw bacc program (no TileContext): one DMA in (packed int8 [128,768]),
~12 DVE ops + 4 GPSIMD mask ops with two handoff semaphores, one DMA out.
"""

import numpy as np

B = 262144
N_CORES = 8
PER_CORE = B // N_CORES  # 32768
P = 128
F = PER_CORE // P  # 256

_CACHE = {}

MASK_ENGINE = "gpsimd"  # engine computing the opcode masks


def _register_custom_ops():
    """Register the three fused ops in concourse.dve_ops' runtime registry."""
    import concourse.dve_ops as dve_ops
    from concourse.dve_spec import (
        AluOp,
        Bin,
        C0,
        C1,
        Spec,
        Src0,
        Src1,
        Zero,
        lower,
        maxx,
        select,
        spec_leaves,
    )
    from concourse.dve_spec import Src1 as _Src1
    from concourse.dve_uop import DveOpSpec

    existing = {op.name: op for op in dve_ops.OPS}

    def reg(name, spec):
        if name in existing:
            return existing[name]
        row = dve_ops._CUSTOM_DVE_ROW_BASE + len(dve_ops.OPS)
        assert row < 0x20
        dve_ops._SUB_OPCODE_FOR_NAME[name] = row
        shas = {}
        for ver in ("v3", "v4"):
            try:
                s = DveOpSpec(
                    name=name,
                    opcode=row,
                    uops=lower(spec, ver=ver),
                    rd1_en=_Src1 in spec_leaves(spec),
                )
                shas[ver] = s.sha(ver)
            except Exception:
                pass  # v4 lowering may differ; TRN2 needs v3 only
        op = dve_ops.DveOp(name, spec, subdim=False, uops_sha=shas)
        dve_ops.OPS.append(op)
        dve_ops.CUSTOM_DVE_SPECS[name] = spec
        return op

    f32 = np.float32

    # FAM: out = |a|*b if a<0 else |a|+b   (sign of a carries [opcode==2])
    def _fam_ref(in0, in1, c0, c1, c2):
        a = in0.astype(f32)
        bv = in1.astype(f32)
        av = np.abs(a)
        return np.where(a < 0, (av * bv).astype(f32), (av + bv).astype(f32))

    av = maxx(Src0, Zero - Src0)
    fam = reg(
        "MOE_FAM",
        Spec(
            body=select(Src0 < Zero, av * Src1, av + Src1),
            reference=_fam_ref,
        ),
    )

    # FASTZ: out = (|b| + (bits(b) & bits(inf))) * 0.5  == z * 2^e
    # |b| keeps Z nonzero on the don't-care lanes where b is sign-packed.
    def _fastz_ref(in0, in1, c0, c1, c2):
        bv = in0.astype(f32)
        pow2 = (bv.view(np.int32) & np.int32(0x7F800000)).view(f32)
        return ((np.abs(bv) + pow2) * f32(c1)).astype(f32)

    fastz = reg(
        "MOE_FASTZ",
        Spec(
            body=Bin(
                AluOp.MULTIPLY,
                Bin(
                    AluOp.ADD,
                    maxx(Src0, Zero - Src0),
                    Bin(AluOp.BITWISE_AND, Src0, C0),
                ),
                C1,
            ),
            reference=_fastz_ref,
        ),
    )

    # NEWTON2B: two Newton steps of Y <- (2 - b*Y)*Y  (Src0=b, Src1=Y0)
    def _newton2b_ref(in0, in1, c0, c1, c2):
        bv = in0.astype(f32)
        y = in1.astype(f32)
        for _ in range(2):
            u = (bv * y).astype(f32)
            v = (f32(c0) - u).astype(f32)
            y = (v * y).astype(f32)
        return y

    y1 = (C0 - Src0 * Src1) * Src1
    y2 = (C0 - Src0 * y1) * y1
    newton2b = reg("MOE_NEWTON2B", Spec(body=y2, reference=_newton2b_ref))

    return fam, fastz, newton2b


def _build_program():
    from concourse import bacc, mybir
    from concourse.dve_ops import RECIPROCAL_APPROX_NR

    fam, fastz, newton2b = _register_custom_ops()

    Alu = mybir.AluOpType
    dt = mybir.dt

    nc = bacc.Bacc("TRN2", target_bir_lowering=False, debug=False)

    # Drop the Bass.__init__ const-AP memsets and the all-engine entry
    # barrier: this kernel uses no const APs, and NRT resets semaphore state
    # per execution (verified by repeat-run correctness), so the barrier only
    # stalls the DMA behind the slowest engine's boot (~1.4us).
    for f in nc.m.functions:
        for blk in f.blocks:
            keep = []
            for ins in blk.instructions:
                if ins.opcode in ("Drain", "EventSemaphore"):
                    continue
                if ins.opcode == "Memset":
                    outs = ins.outs
                    if outs and "const-" in str(outs[0]):
                        continue
                keep.append(ins)
            blk.instructions[:] = keep

    abo8 = nc.declare_dram_parameter("abo8", [P, 3 * F], dt.int8, isOutput=False)
    out = nc.declare_dram_parameter("out", [P, F], dt.float32, isOutput=True)

    def sb(name, dtype, shape=(P, F)):
        return nc.alloc_sbuf_tensor(name, list(shape), dtype).ap()

    tin = sb("tin", dt.int8, (P, 3 * F))
    a8 = tin[:, 0:F]
    b8 = tin[:, F : 2 * F]
    o8 = tin[:, 2 * F : 3 * F]

    fres = sb("fres", dt.float32)
    mres = sb("mres", dt.float32)
    iand8 = sb("iand8", dt.int8)
    ior8 = sb("ior8", dt.int8)
    ixor8 = sb("ixor8", dt.int8)
    zt = sb("zt", dt.float32)
    yf = sb("yf", dt.float32)
    rv = sb("rv", dt.float32)
    wa = sb("wa", dt.float32, (P, 4))
    wb = sb("wb", dt.float32, (P, 4))
    masks = [sb(f"m{k}", dt.uint8) for k in range(3, 7)]
    sqs = [sb(f"sq{k}", dt.float32) for k in range(3, 7)]
    # [P,1] broadcast operand holding the +inf bit pattern 0x7F800000
    # (an inf immediate would serialize to null in BIR JSON; memset packs bits)
    infc = sb("infc", dt.float32, (P, 1))
    # [P,1] bias tiles for ACT mask ops (framework const-APs were stripped)
    negk = [sb(f"negk{k}", dt.float32, (P, 1)) for k in range(3, 7)]
    onec = sb("onec", dt.float32, (P, 1))
    warm = sb("warm", dt.float32, (P, 1))

    dsem = nc.alloc_semaphore("dsem")
    msem = nc.alloc_semaphore("msem")
    asem = nc.alloc_semaphore("asem")
    vsem = nc.alloc_semaphore("vsem")

    # --- SP: input DMA, then wait for compute and write back ---
    nc.sync.dma_start(out=tin[:], in_=abo8[:]).then_inc(dsem, 16)
    nc.sync.wait_ge(vsem, 1)
    nc.sync.dma_start(out=out[:], in_=fres[:]).then_inc(dsem, 16)
    nc.sync.wait_ge(dsem, 32)

    # --- ACT: masks m_k = relu(1 - (o-k)^2), exact {0.0, 1.0} on int
    # opcodes. A dummy activation first so the ACT function-table set loads
    # during boot, overlapped with the input DMA flight.
    Act = mybir.ActivationFunctionType
    a_ = nc.scalar
    a_.activation(warm[:], onec[:], Act.Relu, bias=onec[:], scale=1.0)
    a_.wait_ge(msem, 1)  # bias tiles ready (DVE memsets)
    a_.wait_ge(dsem, 16)
    for i in range(4):
        a_.activation(sqs[i][:], o8, Act.Square, bias=negk[i][:], scale=1.0)
        a_.activation(
            masks[i][:], sqs[i][:], Act.Relu, bias=onec[:], scale=-1.0
        ).then_inc(asem, 1)

    # --- DVE: experts + recip + routing (GpSimd shares an exclusive SBUF
    # port with DVE, so offloading elementwise work there blocks DVE) ---
    v = nc.vector
    v.memset(infc[:], float(np.inf))  # during boot/DMA: free
    for i, k in enumerate(range(3, 7)):
        v.memset(negk[i][:], float(-k))
    v.memset(onec[:], 1.0).then_inc(msem, 1)
    # warm the custom-op rows on tiny tiles while the DMA is in flight
    v.memset(wa[:], 2.0)
    v._custom_dve(fam, out=wb[:], in0=wa[:], in1=wa[:])
    v._custom_dve(fastz, out=wb[:], in0=wa[:], s0=infc[:], s1=0.5)
    v.reciprocal_approx_fast(wb[:], wa[:])
    v._custom_dve(newton2b, out=wb[:], in0=wa[:], in1=wa[:], s0=2.0)
    v.wait_ge(dsem, 16)
    # F = |a| + b  (opc 0,1: b sign-packed)  or |a|*b (opc 2: a sign-packed)
    v._custom_dve(fam, out=fres[:], in0=a8, in1=b8)
    v.tensor_tensor(iand8[:], a8, b8, Alu.bitwise_and)
    v.tensor_tensor(ior8[:], a8, b8, Alu.bitwise_or)
    v.tensor_tensor(ixor8[:], a8, b8, Alu.bitwise_xor)
    # recip expert: Z, Y0 ~= 1/Z (~51 ULP seed; two Newton steps contract the
    # seed-vs-table difference by ~4e0^3 ~ 0.1, leaving ~1e-8 norm error)
    v._custom_dve(fastz, out=zt[:], in0=b8, s0=infc[:], s1=0.5)
    v.reciprocal_approx_fast(yf[:], zt[:])
    v._custom_dve(newton2b, out=rv[:], in0=b8, in1=yf[:], s0=2.0)
    # routing: predicated overwrites of fres (masks from ACT)
    for i, data in enumerate([iand8, ior8, ixor8, rv]):
        v.wait_ge(asem, i + 1)
        ins = v.copy_predicated(fres[:], masks[i][:], data[:])
    ins.then_inc(vsem, 1)

    nc.compile()
    return nc


def _get_program():
    if "nc" not in _CACHE:
        _CACHE["nc"] = _build_program()
    return _CACHE["nc"]


def _pack_inputs(a, b, opcode):
    """Shard + sign-pack + concat into one int8 [P, 3F] tensor per core."""
    a8 = a.astype(np.int8)
    b8 = b.astype(np.int8)
    o8 = opcode.astype(np.int8)
    a8 = np.where(o8 == 2, -a8, a8).reshape(N_CORES, P, F)
    b8 = np.where(o8 == 1, -b8, b8).reshape(N_CORES, P, F)
    o8 = o8.reshape(N_CORES, P, F)
    return [
        np.ascontiguousarray(np.concatenate([a8[i], b8[i], o8[i]], axis=1))
        for i in range(N_CORES)
    ]


def run(a, b, opcode, trace=False):
    from concourse.bass_utils import run_bass_kernel_spmd

    nc = _get_program()
    in_maps = [{"abo8": m} for m in _pack_inputs(a, b, opcode)]
    res = run_bass_kernel_spmd(nc, in_maps, list(range(N_CORES)), trace=trace)
    out = np.concatenate([r["out"].reshape(-1) for r in res.results])
    return out.astype(np.float32, copy=False), res


def kernel(a, b, opcode, and_table, or_table, xor_table, recip_val):
    out, _ = run(np.asarray(a), np.asarray(b), np.asarray(opcode))
    return out

